# revision 61
# baseline (speedup 1.0000x reference)
"""GQA causal attention (RoPE) on 8 Trainium2 NeuronCores.

Sharding (tensor-parallel over heads, per the hint):
  core c owns q-heads {2c, 2c+1} and kv-head c//2.
  Each core computes its 2 heads' attention over the full sequence and a
  partial output projection out_c.T = wo[:, 128c:128c+128] @ att_c  (shape
  [1024, 4096]); the final all-reduce over cores is the host-side unshard.

Device-side per core (v5 — phase-pipelined, dense PE stream):
  Phase n = attention groups of chunk n (SQ=512 q-cols; group = one key
  block j x both heads).  Interleaved INTO the group stream of phase n:
    - rope(n+1) on DVE/gpsimd (projections were done one phase earlier),
    - endgame(n-1): denominator broadcast (two K=1 matmuls), fast
      reciprocal, gpsimd normalize, 8 wo matmuls + PSUM evacuation split
      DVE/scalar, paired [128,2,512] stores,
    - proj(n+2) into the single rotating PSUM pair-bank,
    - v^T(n) transposes (one 4-block PSUM tile, one DVE evacuation).
  The PE therefore never waits on rope/exp at chunk boundaries and stays
  HAM-warm; scalar exp (the co-critical engine) is trimmed on diagonal
  groups via strided APs.

  PSUM banks: sc 2x[128,2,512] (4) + pp2 [128,2,512] (2, shared by
  proj/wo/bc) + av0/av1 (2) = 8.
"""
import numpy as np
import ml_dtypes
from contextlib import ExitStack

import concourse.bacc as bacc
import concourse.tile as tile
import concourse.mybir as mybir
from concourse.bass_utils import run_bass_kernel_spmd

DIM = 1024
N_HEADS = 16
N_KV = 4
HD = 64
SEQ = 4096
NCORES = 8

SQ = 512            # query-chunk (free dim of score blocks)
SK = 128            # key-chunk (partition dim of score blocks)
NQ = SEQ // SQ      # 8
NR = DIM // 128     # 8 contraction chunks for projections
NJ = SEQ // SK      # 32 key chunks
MASKVAL = -1.0e6

f32 = mybir.dt.float32
f32r = mybir.dt.float32r
bf16 = mybir.dt.bfloat16
FT = mybir.ActivationFunctionType

_CACHE = {}


def _emit(nc):
    xT = nc.dram_tensor("xT", [DIM, SEQ], bf16, kind="ExternalInput").ap()
    wq_l = nc.dram_tensor("wq_l", [128, DIM], bf16, kind="ExternalInput").ap()
    wkv_l = nc.dram_tensor("wkv_l", [128, DIM], bf16, kind="ExternalInput").ap()
    wo_l = nc.dram_tensor("wo_l", [128, DIM], bf16, kind="ExternalInput").ap()
    cos4_d = nc.dram_tensor("cos4", [128, SEQ], bf16, kind="ExternalInput").ap()
    sin4_d = nc.dram_tensor("sin4", [128, SEQ], bf16, kind="ExternalInput").ap()
    mask_d = nc.dram_tensor("mask", [128, 2, 4 * SQ], bf16, kind="ExternalInput").ap()
    id_d = nc.dram_tensor("ident", [128, 128], bf16, kind="ExternalInput").ap()
    psw_d = nc.dram_tensor("pswap", [128, 128], bf16, kind="ExternalInput").ap()
    sel_d = nc.dram_tensor("selr", [128, 256], f32r, kind="ExternalInput").ap()
    out_d = nc.dram_tensor("out", [128, NR, SEQ], bf16, kind="ExternalOutput").ap()

    with tile.TileContext(nc) as tc, ExitStack() as ctx:
        const = ctx.enter_context(tc.tile_pool(name="const", bufs=1))
        main = ctx.enter_context(tc.tile_pool(name="main", bufs=1))

        wq_sb = const.tile([128, DIM], bf16)
        wkv_sb = const.tile([128, DIM], bf16)
        wo_sb = const.tile([128, DIM], bf16)
        cos_sb = const.tile([128, SEQ], bf16)
        sin_sb = const.tile([128, SEQ], bf16)
        msk_sb = const.tile([128, 2, 4 * SQ], bf16)
        id_sb = const.tile([128, 128], bf16)
        psw_sb = const.tile([128, 128], bf16)
        sel_sb = const.tile([128, 256], f32r)

        qrot = main.tile([128, SEQ], bf16)      # 2 heads d-major (rope'd)
        krot = main.tile([128, SEQ], bf16)      # k duplicated in both halves
        v_sb = main.tile([HD, SEQ], bf16)       # v d-major
        vt = main.tile([128, NJ, HD + 1], bf16)  # v^T + ones column

        with (
            tc.tile_pool(name="xp", bufs=16) as xp,
            tc.tile_pool(name="rp", bufs=2) as rp,
            tc.tile_pool(name="ep", bufs=12) as ep,
            tc.tile_pool(name="aup", bufs=2) as aup,
            tc.tile_pool(name="asp", bufs=2) as asp,
            tc.tile_pool(name="owp", bufs=3) as owp,
            tc.tile_pool(name="pp", bufs=1, space="PSUM") as pp,
            tc.tile_pool(name="sp", bufs=2, space="PSUM") as sp,
            tc.tile_pool(name="ap", bufs=1, space="PSUM") as ap,
        ):
            xts = {}        # chunk -> list of 8 xt tiles
            projp = {}      # chunk -> (pq_ap, pkv_ap, pp2 tile) PSUM views
            aus = {}        # chunk -> (au0, au1) staged raw AV
            attSs = {}      # chunk -> attS tile (normalized, stacked)

            def load_xt(n, engs):
                ts = []
                for r in range(NR):
                    t = xp.tile([128, SQ], bf16, tag="xt", name=f"xt_{n}_{r}")
                    engs[r % len(engs)].dma_start(
                        t[:], xT[128 * r:128 * (r + 1), n * SQ:(n + 1) * SQ])
                    ts.append(t)
                xts[n] = ts

            def proj_half(n, pq_ap, pkv_ap, half):
                for r in range(4 * half, 4 * half + 4):
                    t = xts[n][r]
                    nc.tensor.matmul(pq_ap, wq_sb[:, 128 * r:128 * (r + 1)], t,
                                     start=(r == 0), stop=(r == NR - 1))
                    nc.tensor.matmul(pkv_ap, wkv_sb[:, 128 * r:128 * (r + 1)], t,
                                     start=(r == 0), stop=(r == NR - 1))
                if half == 1:
                    del xts[n]

            def proj_quarter(n, q):
                pq_ap, pkv_ap = projp[n]
                for r in range(2 * q, 2 * q + 2):
                    t = xts[n][r]
                    nc.tensor.matmul(pq_ap, wq_sb[:, 128 * r:128 * (r + 1)], t,
                                     start=(r == 0), stop=(r == NR - 1))
                    nc.tensor.matmul(pkv_ap, wkv_sb[:, 128 * r:128 * (r + 1)], t,
                                     start=(r == 0), stop=(r == NR - 1))
                if q == 3:
                    del xts[n]

            ropes = {}      # chunk -> (aq, cqb, bq_dst) awaiting rope_fin

            def rope(n, pq_ap, pkv_ap):
                # DVE: kmul,kmul,vcopy,cqmul,aqmul,kadd; the q rotate-half
                # goes through the PE (rope_fin) instead of gpsimd DMAs.
                s0 = n * SQ
                ak = rp.tile([128, SQ], f32, tag="ta", name=f"ak_{n}")
                ck = rp.tile([128, SQ], f32, tag="tc", name=f"ck_{n}")
                bk = rp.tile([128, SQ], f32, tag="tb", name=f"bk_{n}")
                nc.vector.tensor_mul(ak[64:128, :], pkv_ap[64:128, :],
                                     cos_sb[64:128, s0:s0 + SQ])
                nc.vector.tensor_mul(ck[64:128, :], pkv_ap[64:128, :],
                                     sin_sb[64:128, s0:s0 + SQ])
                nc.vector.tensor_copy(v_sb[:, s0:s0 + SQ], pkv_ap[0:64, :])
                cqb = rp.tile([128, SQ], bf16, tag="tq", name=f"cqb_{n}")
                aq = rp.tile([128, SQ], f32, tag="ta", name=f"aq_{n}")
                nc.vector.tensor_mul(cqb[:], pq_ap, sin_sb[:, s0:s0 + SQ])
                nc.vector.tensor_mul(aq[:], pq_ap, cos_sb[:, s0:s0 + SQ])
                nc.gpsimd.dma_start(bk[64:96, :], ck[96:128, :])
                nc.gpsimd.dma_start(bk[96:128, :], ck[64:96, :])
                nc.vector.tensor_add(krot[64:128, s0:s0 + SQ], ak[64:128, :],
                                     bk[64:128, :])
                nc.gpsimd.dma_start(krot[0:64, s0:s0 + SQ], krot[64:128, s0:s0 + SQ])
                ropes[n] = (aq, cqb, pq_ap)

            def rope_fin(n, bq_dst=None):
                # q rotate-half: PE permutation matmul, then the add on DVE
                s0 = n * SQ
                aq, cqb, pq_ap = ropes.pop(n)
                if bq_dst is None:
                    bq_dst = pq_ap
                nc.tensor.matmul(bq_dst, psw_sb[:, :], cqb[:],
                                 start=True, stop=True)
                nc.vector.tensor_add(qrot[:, s0:s0 + SQ], aq[:], bq_dst)

            def vtr(n):
                # 4 v^T transposes into one PSUM tile, one DVE evacuation
                pt4 = sp.tile([128, 4, HD], bf16, tag="sc", name=f"pt4_{n}")
                for t in range(4):
                    j = 4 * n + t
                    nc.tensor.transpose(pt4[:, t, :], v_sb[:, SK * j:SK * (j + 1)],
                                        id_sb[0:HD, 0:HD])
                nc.vector.tensor_copy(vt[:, 4 * n:4 * n + 4, 0:HD], pt4[:, :, :])

            def endgame_bc(c, tail=False):
                # denominators broadcast via two K=1 matmuls (no gather),
                # fast reciprocal, normalize split DVE/gpsimd, stack via DMA
                au0_t, au1_t = aus[c]
                bc = pp.tile([128, 2, SQ], f32, tag="pp2", name=f"bc_{c}")
                nc.tensor.matmul(bc[:, 0, :], sel_sb[64:65, 0:128],
                                 au0_t[64:65, :], start=True, stop=False)
                nc.tensor.matmul(bc[:, 0, :], sel_sb[64:65, 128:256],
                                 au1_t[64:65, :], start=False, stop=True)
                rb = asp.tile([128, SQ], f32, tag="rb", name=f"rb_{c}")
                nc.vector.reciprocal_approx_fast(rb[:], bc[:, 0, :])
                rb1 = asp.tile([HD, SQ], f32, tag="rb1", name=f"rb1_{c}")
                attS = asp.tile([128, SQ], bf16, tag="attS", name=f"attS_{c}")
                att1 = asp.tile([HD, SQ], bf16, tag="att1", name=f"att1_{c}")
                nc.gpsimd.dma_start(rb1[:], rb[64:128, :])
                nc.vector.tensor_mul(attS[0:HD, :], au0_t[0:HD, :].bitcast(f32),
                                     rb[0:HD, :])
                eng = nc.vector if tail else nc.gpsimd
                eng.tensor_mul(att1[:, :], au1_t[0:HD, :].bitcast(f32),
                               rb1[:, :])
                nc.gpsimd.dma_start(attS[64:128, :], att1[:, :])
                attSs[c] = attS

            # groups of chunk n+1 pre-run (scores+exp) at the end of phase n
            STEAL = [2, 3, 4, 5, 6, 7, 10]
            avs = {}        # chunk -> [av0, av1] PSUM accumulators
            pend = []       # (chunk, j, et, dd) exp'd groups awaiting AV

            def sgrp(c, j):
                s0c = c * SQ
                delta = SK * j - s0c
                dd = max(0, delta)
                sc = sp.tile([128, 2, SQ], f32, tag="sc", name=f"sc_{c}_{j}")
                for h in (0, 1):
                    nc.tensor.matmul(
                        sc[:, h, dd:SQ],
                        krot[64 * h:64 * h + 64, SK * j:SK * (j + 1)],
                        qrot[64 * h:64 * h + 64, s0c + dd:s0c + SQ],
                        start=True, stop=(delta < 0),
                        skip_group_check=(delta >= 0))
                if delta >= 0:
                    db = (delta // SK) * SQ + dd
                    nc.tensor.matmul(sc[:, :, delta:delta + SK],
                                     id_sb[:], msk_sb[:, :, db:db + SK],
                                     start=False, stop=True,
                                     skip_group_check=True)
                et = ep.tile([128, 2, SQ], bf16, tag="et", name=f"et_{c}_{j}")
                nc.scalar.activation(et[:, :, dd:SQ], sc[:, :, dd:SQ],
                                     FT.Exp, scale=0.125)
                pend.append((c, j, et, dd))

            def flush():
                c_, j_, et_, dd_ = pend.pop(0)
                for h in (0, 1):
                    nc.tensor.matmul(
                        avs[c_][h][:, dd_:SQ], vt[:, j_, 0:HD + 1],
                        et_[:, h, dd_:SQ],
                        start=(j_ == 0), stop=(j_ == 4 * (c_ + 1) - 1))

            def stage_av(n, av, ev):
                au0_t = aup.tile([HD + 1, SQ], f32r, tag="au0", name=f"au0_{n}")
                au1_t = aup.tile([HD + 1, SQ], f32r, tag="au1", name=f"au1_{n}")
                if ev == "s":
                    nc.scalar.activation(au0_t[:], av[0][:], FT.Copy)
                else:
                    nc.vector.tensor_copy(au0_t[:], av[0][:])
                nc.vector.tensor_copy(au1_t[:], av[1][:])
                aus[n] = (au0_t, au1_t)

            def wo_pair(c, p, evac, pool=None):
                sk0 = c * SQ
                pool = pool or pp
                tg = "pp2" if pool is pp else "sc"
                pw = pool.tile([128, 2, SQ], f32, tag=tg, name=f"pw_{c}_{p}")
                for i in (0, 1):
                    m = 2 * p + i
                    nc.tensor.matmul(pw[:, i, :], wo_sb[:, 128 * m:128 * (m + 1)],
                                     attSs[c][:, :], start=True, stop=True)
                ow = owp.tile([128, 2, SQ], bf16, tag="ow", name=f"ow_{c}_{p}")
                if evac == "s":
                    nc.scalar.activation(ow[:, :, :], pw[:, :, :], FT.Copy)
                else:
                    nc.vector.tensor_copy(ow[:, :, :], pw[:, :, :])
                eng = nc.sync if p % 2 == 0 else nc.scalar
                eng.dma_start(out_d[:, 2 * p:2 * p + 2, sk0:sk0 + SQ], ow[:, :, :])

            # ---------------- prologue ----------------
            # sync/gpsimd queues feed proj(0,1) x-tiles; scalar carries
            # consts + trig(0..2) + xt(2); the ones column is a DVE memset.
            nc.sync.dma_start(wq_sb[:, 0:256], wq_l[:, 0:256])
            nc.gpsimd.dma_start(wkv_sb[:, 0:256], wkv_l[:, 0:256])
            # chunks 0+1 as paired [128,2,512] tiles: 2KB descriptor rows
            # double the effective per-queue DMA throughput
            q3 = [nc.sync, nc.gpsimd, nc.scalar]
            xts[0] = []
            xts[1] = []
            for r in range(NR):
                if r == 2:
                    nc.sync.dma_start(wq_sb[:, 256:DIM], wq_l[:, 256:DIM])
                    nc.gpsimd.dma_start(wkv_sb[:, 256:DIM], wkv_l[:, 256:DIM])
                tp_ = xp.tile([128, 2, SQ], bf16, tag="xt", name=f"xtp_{r}")
                q3[r % 3].dma_start(tp_[:, :, :],
                                    xT[128 * r:128 * (r + 1), 0:2 * SQ])
                xts[0].append(tp_[:, 0, :])
                xts[1].append(tp_[:, 1, :])
            nc.scalar.dma_start(cos_sb[:, 0:3 * SQ], cos4_d[:, 0:3 * SQ])
            nc.scalar.dma_start(sin_sb[:, 0:3 * SQ], sin4_d[:, 0:3 * SQ])
            nc.scalar.dma_start(id_sb[:], id_d[:])
            nc.scalar.dma_start(psw_sb[:], psw_d[:])
            nc.sync.dma_start(msk_sb[:, 0, :], mask_d[:, 0, :])
            nc.gpsimd.dma_start(msk_sb[:, 1, :], mask_d[:, 1, :])
            load_xt(2, [nc.scalar])
            nc.scalar.dma_start(wo_sb[:], wo_l[:])
            nc.scalar.dma_start(sel_sb[:], sel_d[:])
            nc.vector.memset(vt[:, :, HD:HD + 1], 1.0)

            # PE warm-up: junk matmuls on the first-arriving weight slice keep
            # the tensor engine continuously busy through the DMA-bound load
            # window so the p-state ramp reaches full clock before proj(0).
            wrm = sp.tile([128, 2, SQ], f32, tag="sc", name="wrm")
            for _ in range(40):
                nc.tensor.matmul(wrm[:, 0, 0:128], wq_sb[:, 0:128],
                                 wq_sb[0:128, 0:128], start=True, stop=True)

            pp0 = pp.tile([128, 2, SQ], f32, tag="pp2", name="pp2_0")
            projp[0] = (pp0[:, 0, :], pp0[:, 1, :])
            proj_half(0, *projp[0], 0)
            proj_half(0, *projp[0], 1)
            rope(0, *projp[0])
            boot0 = ap.tile([128, SQ], f32, tag="av0", name="boot0")
            boot1 = ap.tile([128, SQ], f32, tag="av1", name="boot1")
            projp[1] = (boot0[:, :], boot1[:, :])
            proj_half(1, *projp[1], 0)
            proj_half(1, *projp[1], 1)
            rope_fin(0)
            vtr(0)

            # ---------------- phases ----------------
            for n in range(NQ):
                s0 = n * SQ
                nsk = 4 * (n + 1)
                if n > 0:
                    vtr(n)
                if n + 1 < NQ:
                    rope(n + 1, *projp.pop(n + 1))
                if n + 3 < NQ:
                    load_xt(n + 3, [nc.sync])
                    t3 = (n + 3) * SQ
                    nc.sync.dma_start(cos_sb[:, t3:t3 + SQ], cos4_d[:, t3:t3 + SQ])
                    nc.gpsimd.dma_start(sin_sb[:, t3:t3 + SQ], sin4_d[:, t3:t3 + SQ])

                # attachment thunks keyed by group index (leftovers pile at end)
                attach = {}

                def att_at(j, th):
                    attach.setdefault(j, []).append(th)

                # interleave pieces spread evenly across the group stream so
                # the 2-deep sc/exp pipeline never drains behind a pile-up
                pieces = []
                if n == 0:
                    def fin1():
                        bq1 = pp.tile([128, 2, SQ], f32, tag="pp2", name="bq1")
                        rope_fin(1, bq1[:, 0, :])

                    pieces.append(fin1)
                elif n + 1 < NQ:
                    pieces.append(lambda v=n + 1: rope_fin(v))
                if n >= 1:
                    c = n - 1
                    ev = "s" if n <= 3 else "v"
                    pieces.append(lambda c=c: endgame_bc(c))
                    pieces.append(lambda c=c: wo_pair(c, 0, "v"))
                    pieces.append(lambda c=c: wo_pair(c, 1, "v"))
                    pieces.append(lambda c=c, ev=ev: wo_pair(c, 2, ev))
                    pieces.append(lambda c=c, ev=ev: wo_pair(c, 3, ev))
                m = n + 2
                if m < NQ:
                    def proj_q(m=m, q=0):
                        pt = pp.tile([128, 2, SQ], f32, tag="pp2", name=f"pp2_{m}")
                        projp[m] = (pt[:, 0, :], pt[:, 1, :])
                        proj_quarter(m, q)

                    pieces.append(proj_q)
                    for q in range(1, 4):
                        pieces.append(lambda m=m, q=q: proj_quarter(m, q))
                G = nsk - (STEAL[n - 1] if n else 0)
                lo = 1 if n == 0 else 3
                for k, th in enumerate(pieces):
                    pos = lo + (k * max(0, G - 1 - lo)) // max(1, len(pieces) - 1) \
                        if len(pieces) > 1 else lo
                    att_at(min(pos, G - 1), th)

                avs[n] = [ap.tile([HD + 1, SQ], f32, tag=f"av{h}",
                                  name=f"av{h}_{n}") for h in (0, 1)]

                for idx, j in enumerate(range(STEAL[n - 1] if n else 0, nsk)):
                    sgrp(n, j)
                    if len(pend) > 2:
                        flush()
                    for th in attach.pop(idx, []):
                        th()
                while pend:
                    flush()

                stage_av(n, avs[n], "s" if (n <= 3 or n == NQ - 1) else "v")

                # leftovers (small phases): preserve order
                for idx in sorted(attach):
                    for th in attach[idx]:
                        th()

                # steal: next chunk's first score groups + exps run here,
                # filling this phase's scalar idle; their AV flushes happen
                # in the next phase once its accumulator bank opens.
                if n + 1 < NQ:
                    for j in range(STEAL[n]):
                        sgrp(n + 1, j)

            # ---------------- tail: endgame of last chunk ----------------
            # wo pairs alternate between the pp2 and (now idle) sc PSUM
            # rotations and between DVE/scalar evacuation so they pipeline.
            c = NQ - 1
            endgame_bc(c)
            wo_pair(c, 0, "v")
            wo_pair(c, 1, "s", pool=sp)
            wo_pair(c, 2, "v")
            wo_pair(c, 3, "s", pool=sp)


def _build():
    if "nc" in _CACHE:
        return _CACHE["nc"]
    nc = bacc.Bacc("TRN2", target_bir_lowering=False, debug=False, num_devices=NCORES)
    _emit(nc)
    nc.compile()
    _CACHE["nc"] = nc
    return nc


def _host_inputs(x, freqs_cos, freqs_sin, wq, wk, wv, wo):
    x = np.asarray(x, np.float32)
    freqs_cos = np.asarray(freqs_cos, np.float32)
    freqs_sin = np.asarray(freqs_sin, np.float32)
    wq = np.asarray(wq, np.float32)
    wk = np.asarray(wk, np.float32)
    wv = np.asarray(wv, np.float32)
    wo = np.asarray(wo, np.float32)

    xT = np.ascontiguousarray(x[0].T).astype(ml_dtypes.bfloat16)   # [1024, 4096]
    cosT = freqs_cos.T                                             # [32, 4096]
    sinT = freqs_sin.T
    cos4 = np.ascontiguousarray(np.tile(cosT, (4, 1))).astype(
        ml_dtypes.bfloat16)                                        # [128, 4096]
    sin4 = np.ascontiguousarray(
        np.concatenate([sinT, -sinT, sinT, -sinT], axis=0)).astype(
        ml_dtypes.bfloat16)

    # q rotate-half partition permutation (swap 32-row blocks in each half)
    pswap = np.zeros((128, 128), dtype=ml_dtypes.bfloat16)
    for mm in range(128):
        kk = mm + 32 if (mm % 64) < 32 else mm - 32
        pswap[kk, mm] = 1.0

    # diagonal-block causal masks for delta in {0,128,256,384}
    p = np.arange(SK)[:, None]
    f = np.arange(SQ)[None, :]
    mask = np.concatenate(
        [np.where(SK * d + p <= f, 0.0, MASKVAL) for d in range(4)],
        axis=1).astype(ml_dtypes.bfloat16)                         # [128, 2048]
    mask = np.ascontiguousarray(
        np.repeat(mask[:, None, :], 2, axis=1))                    # [128, 2, 2048]

    ident = np.eye(128, dtype=ml_dtypes.bfloat16)
    # two K=1 selector rows at partition 64: cols 0:128 broadcast head-0's
    # denominator to partitions 0:64, cols 128:256 head-1's to 64:128
    selr = np.zeros((128, 256), dtype=np.float32)
    selr[64, 0:64] = 1.0
    selr[64, 192:256] = 1.0

    perm = np.concatenate([np.arange(0, HD, 2), np.arange(1, HD, 2)])

    def fold(w):  # [128(m), 1024(d)] -> lhsT layout [128(p), 8r*128+m]
        return np.ascontiguousarray(
            w.reshape(128, NR, 128).transpose(2, 1, 0).reshape(128, DIM)
        ).astype(ml_dtypes.bfloat16)

    in_maps = []
    for c in range(NCORES):
        g = c // 2
        wq_c = wq[128 * c:128 * (c + 1)].reshape(2, HD, DIM)[:, perm, :].reshape(128, DIM)
        wk_g = wk[HD * g:HD * (g + 1)][perm]
        wv_g = wv[HD * g:HD * (g + 1)]
        wkv_c = np.concatenate([wv_g, wk_g], axis=0)        # v rows 0:64, k rows 64:128
        wo_c = np.ascontiguousarray(wo[:, 128 * c:128 * (c + 1)].T).astype(
            ml_dtypes.bfloat16)                              # [128(j), 1024(o)]
        in_maps.append({
            "xT": xT,
            "wq_l": fold(wq_c),
            "wkv_l": fold(wkv_c),
            "wo_l": wo_c,
            "cos4": cos4,
            "sin4": sin4,
            "mask": mask,
            "ident": ident,
            "pswap": pswap,
            "selr": selr,
        })
    return in_maps


def kernel(x, freqs_cos, freqs_sin, wq, wk, wv, wo, _trace=False, _trace_kwargs=None):
    nc = _build()
    in_maps = _host_inputs(x, freqs_cos, freqs_sin, wq, wk, wv, wo)
    kw = {}
    if _trace:
        kw.update(trace=True, **(_trace_kwargs or {}))
    res = run_bass_kernel_spmd(nc, in_maps, core_ids=list(range(NCORES)), **kw)
    acc = np.zeros((128, NR, SEQ), np.float32)
    for c in range(NCORES):
        acc += res.results[c]["out"].astype(np.float32)
    full = np.ascontiguousarray(acc.transpose(1, 0, 2)).reshape(DIM, SEQ)
    out = np.ascontiguousarray(full.T).reshape(1, SEQ, DIM)
    if _trace:
        kernel._last_results = res
    return out


# revision 62
# speedup vs baseline: 1.0105x; 1.0105x over previous
"""GQA causal attention (RoPE) on 8 Trainium2 NeuronCores.

Sharding (tensor-parallel over heads, per the hint):
  core c owns q-heads {2c, 2c+1} and kv-head c//2.
  Each core computes its 2 heads' attention over the full sequence and a
  partial output projection out_c.T = wo[:, 128c:128c+128] @ att_c  (shape
  [1024, 4096]); the final all-reduce over cores is the host-side unshard.

Device-side per core (v5 — phase-pipelined, dense PE stream):
  Phase n = attention groups of chunk n (SQ=512 q-cols; group = one key
  block j x both heads).  Interleaved INTO the group stream of phase n:
    - rope(n+1) on DVE/gpsimd (projections were done one phase earlier),
    - endgame(n-1): denominator broadcast (two K=1 matmuls), fast
      reciprocal, gpsimd normalize, 8 wo matmuls + PSUM evacuation split
      DVE/scalar, paired [128,2,512] stores,
    - proj(n+2) into the single rotating PSUM pair-bank,
    - v^T(n) transposes (one 4-block PSUM tile, one DVE evacuation).
  The PE therefore never waits on rope/exp at chunk boundaries and stays
  HAM-warm; scalar exp (the co-critical engine) is trimmed on diagonal
  groups via strided APs.

  PSUM banks: sc 2x[128,2,512] (4) + pp2 [128,2,512] (2, shared by
  proj/wo/bc) + av0/av1 (2) = 8.
"""
import numpy as np
import ml_dtypes
from contextlib import ExitStack

import concourse.bacc as bacc
import concourse.tile as tile
import concourse.mybir as mybir
from concourse.bass_utils import run_bass_kernel_spmd

DIM = 1024
N_HEADS = 16
N_KV = 4
HD = 64
SEQ = 4096
NCORES = 8

SQ = 512            # query-chunk (free dim of score blocks)
SK = 128            # key-chunk (partition dim of score blocks)
NQ = SEQ // SQ      # 8
NR = DIM // 128     # 8 contraction chunks for projections
NJ = SEQ // SK      # 32 key chunks
MASKVAL = -1.0e6

f32 = mybir.dt.float32
f32r = mybir.dt.float32r
bf16 = mybir.dt.bfloat16
FT = mybir.ActivationFunctionType

_CACHE = {}


def _emit(nc):
    xT = nc.dram_tensor("xT", [DIM, SEQ], bf16, kind="ExternalInput").ap()
    wq_l = nc.dram_tensor("wq_l", [128, DIM], bf16, kind="ExternalInput").ap()
    wkv_l = nc.dram_tensor("wkv_l", [128, DIM], bf16, kind="ExternalInput").ap()
    wo_l = nc.dram_tensor("wo_l", [128, DIM], bf16, kind="ExternalInput").ap()
    cos4_d = nc.dram_tensor("cos4", [128, SEQ], bf16, kind="ExternalInput").ap()
    sin4_d = nc.dram_tensor("sin4", [128, SEQ], bf16, kind="ExternalInput").ap()
    mask_d = nc.dram_tensor("mask", [128, 2, 4 * SQ], bf16, kind="ExternalInput").ap()
    id_d = nc.dram_tensor("ident", [128, 128], bf16, kind="ExternalInput").ap()
    psw_d = nc.dram_tensor("pswap", [128, 128], bf16, kind="ExternalInput").ap()
    sel_d = nc.dram_tensor("selr", [128, 256], f32r, kind="ExternalInput").ap()
    out_d = nc.dram_tensor("out", [128, NR, SEQ], bf16, kind="ExternalOutput").ap()

    with tile.TileContext(nc) as tc, ExitStack() as ctx:
        const = ctx.enter_context(tc.tile_pool(name="const", bufs=1))
        main = ctx.enter_context(tc.tile_pool(name="main", bufs=1))

        wq_sb = const.tile([128, DIM], bf16)
        wkv_sb = const.tile([128, DIM], bf16)
        wo_sb = const.tile([128, DIM], bf16)
        cos_sb = const.tile([128, SEQ], bf16)
        sin_sb = const.tile([128, SEQ], bf16)
        msk_sb = const.tile([128, 2, 4 * SQ], bf16)
        id_sb = const.tile([128, 128], bf16)
        psw_sb = const.tile([128, 128], bf16)
        sel_sb = const.tile([128, 256], f32r)

        qrot = main.tile([128, SEQ], bf16)      # 2 heads d-major (rope'd)
        krot = main.tile([128, SEQ], bf16)      # k duplicated in both halves
        v_sb = main.tile([HD, SEQ], bf16)       # v d-major
        vt = main.tile([128, NJ, HD + 1], bf16)  # v^T + ones column

        with (
            tc.tile_pool(name="xp", bufs=16) as xp,
            tc.tile_pool(name="rp", bufs=2) as rp,
            tc.tile_pool(name="ep", bufs=12) as ep,
            tc.tile_pool(name="aup", bufs=2) as aup,
            tc.tile_pool(name="asp", bufs=2) as asp,
            tc.tile_pool(name="owp", bufs=3) as owp,
            tc.tile_pool(name="pp", bufs=1, space="PSUM") as pp,
            tc.tile_pool(name="sp", bufs=2, space="PSUM") as sp,
            tc.tile_pool(name="ap", bufs=1, space="PSUM") as ap,
        ):
            xts = {}        # chunk -> list of 8 xt tiles
            projp = {}      # chunk -> (pq_ap, pkv_ap, pp2 tile) PSUM views
            aus = {}        # chunk -> (au0, au1) staged raw AV
            attSs = {}      # chunk -> attS tile (normalized, stacked)

            def load_xt(n, engs):
                ts = []
                for r in range(NR):
                    t = xp.tile([128, SQ], bf16, tag="xt", name=f"xt_{n}_{r}")
                    engs[r % len(engs)].dma_start(
                        t[:], xT[128 * r:128 * (r + 1), n * SQ:(n + 1) * SQ])
                    ts.append(t)
                xts[n] = ts

            def proj_half(n, pq_ap, pkv_ap, half):
                for r in range(4 * half, 4 * half + 4):
                    t = xts[n][r]
                    nc.tensor.matmul(pq_ap, wq_sb[:, 128 * r:128 * (r + 1)], t,
                                     start=(r == 0), stop=(r == NR - 1))
                    nc.tensor.matmul(pkv_ap, wkv_sb[:, 128 * r:128 * (r + 1)], t,
                                     start=(r == 0), stop=(r == NR - 1))
                if half == 1:
                    del xts[n]

            def proj_quarter(n, q):
                pq_ap, pkv_ap = projp[n]
                for r in range(2 * q, 2 * q + 2):
                    t = xts[n][r]
                    nc.tensor.matmul(pq_ap, wq_sb[:, 128 * r:128 * (r + 1)], t,
                                     start=(r == 0), stop=(r == NR - 1))
                    nc.tensor.matmul(pkv_ap, wkv_sb[:, 128 * r:128 * (r + 1)], t,
                                     start=(r == 0), stop=(r == NR - 1))
                if q == 3:
                    del xts[n]

            ropes = {}      # chunk -> (aq, cqb, bq_dst) awaiting rope_fin

            def rope(n, pq_ap, pkv_ap):
                # DVE: kmul,kmul,vcopy,cqmul,aqmul,kadd; the q rotate-half
                # goes through the PE (rope_fin) instead of gpsimd DMAs.
                s0 = n * SQ
                ak = rp.tile([128, SQ], f32, tag="ta", name=f"ak_{n}")
                ck = rp.tile([128, SQ], f32, tag="tc", name=f"ck_{n}")
                bk = rp.tile([128, SQ], f32, tag="tb", name=f"bk_{n}")
                nc.vector.tensor_mul(ak[64:128, :], pkv_ap[64:128, :],
                                     cos_sb[64:128, s0:s0 + SQ])
                nc.vector.tensor_mul(ck[64:128, :], pkv_ap[64:128, :],
                                     sin_sb[64:128, s0:s0 + SQ])
                nc.vector.tensor_copy(v_sb[:, s0:s0 + SQ], pkv_ap[0:64, :])
                cqb = rp.tile([128, SQ], bf16, tag="tq", name=f"cqb_{n}")
                aq = rp.tile([128, SQ], f32, tag="ta", name=f"aq_{n}")
                nc.vector.tensor_mul(cqb[:], pq_ap, sin_sb[:, s0:s0 + SQ])
                nc.vector.tensor_mul(aq[:], pq_ap, cos_sb[:, s0:s0 + SQ])
                nc.gpsimd.dma_start(bk[64:96, :], ck[96:128, :])
                nc.gpsimd.dma_start(bk[96:128, :], ck[64:96, :])
                nc.vector.tensor_add(krot[64:128, s0:s0 + SQ], ak[64:128, :],
                                     bk[64:128, :])
                nc.gpsimd.dma_start(krot[0:64, s0:s0 + SQ], krot[64:128, s0:s0 + SQ])
                ropes[n] = (aq, cqb, pq_ap)

            def rope_fin(n, bq_dst=None):
                # q rotate-half: PE permutation matmul, then the add on DVE
                s0 = n * SQ
                aq, cqb, pq_ap = ropes.pop(n)
                if bq_dst is None:
                    bq_dst = pq_ap
                nc.tensor.matmul(bq_dst, psw_sb[:, :], cqb[:],
                                 start=True, stop=True)
                nc.vector.tensor_add(qrot[:, s0:s0 + SQ], aq[:], bq_dst)

            def vtr(n):
                # 4 v^T transposes into one PSUM tile, one DVE evacuation
                pt4 = sp.tile([128, 4, HD], bf16, tag="sc", name=f"pt4_{n}")
                for t in range(4):
                    j = 4 * n + t
                    nc.tensor.transpose(pt4[:, t, :], v_sb[:, SK * j:SK * (j + 1)],
                                        id_sb[0:HD, 0:HD])
                nc.vector.tensor_copy(vt[:, 4 * n:4 * n + 4, 0:HD], pt4[:, :, :])

            def endgame_bc(c, tail=False):
                # denominators broadcast via two K=1 matmuls (no gather),
                # fast reciprocal, normalize split DVE/gpsimd, stack via DMA
                au0_t, au1_t = aus[c]
                bc = pp.tile([128, 2, SQ], f32, tag="pp2", name=f"bc_{c}")
                nc.tensor.matmul(bc[:, 0, :], sel_sb[64:65, 0:128],
                                 au0_t[64:65, :], start=True, stop=False)
                nc.tensor.matmul(bc[:, 0, :], sel_sb[64:65, 128:256],
                                 au1_t[64:65, :], start=False, stop=True)
                rb = asp.tile([128, SQ], f32, tag="rb", name=f"rb_{c}")
                nc.vector.reciprocal_approx_fast(rb[:], bc[:, 0, :])
                rb1 = asp.tile([HD, SQ], f32, tag="rb1", name=f"rb1_{c}")
                attS = asp.tile([128, SQ], bf16, tag="attS", name=f"attS_{c}")
                att1 = asp.tile([HD, SQ], bf16, tag="att1", name=f"att1_{c}")
                nc.gpsimd.dma_start(rb1[:], rb[64:128, :])
                nc.vector.tensor_mul(attS[0:HD, :], au0_t[0:HD, :].bitcast(f32),
                                     rb[0:HD, :])
                eng = nc.vector if tail else nc.gpsimd
                eng.tensor_mul(att1[:, :], au1_t[0:HD, :].bitcast(f32),
                               rb1[:, :])
                nc.gpsimd.dma_start(attS[64:128, :], att1[:, :])
                attSs[c] = attS

            # groups of chunk n+1 pre-run (scores+exp) at the end of phase n
            STEAL = [1, 2, 3, 4, 5, 6, 8]
            avs = {}        # chunk -> [av0, av1] PSUM accumulators
            pend = []       # (chunk, j, et, dd) exp'd groups awaiting AV

            def sgrp(c, j):
                s0c = c * SQ
                delta = SK * j - s0c
                dd = max(0, delta)
                sc = sp.tile([128, 2, SQ], f32, tag="sc", name=f"sc_{c}_{j}")
                for h in (0, 1):
                    nc.tensor.matmul(
                        sc[:, h, dd:SQ],
                        krot[64 * h:64 * h + 64, SK * j:SK * (j + 1)],
                        qrot[64 * h:64 * h + 64, s0c + dd:s0c + SQ],
                        start=True, stop=(delta < 0),
                        skip_group_check=(delta >= 0))
                if delta >= 0:
                    db = (delta // SK) * SQ + dd
                    nc.tensor.matmul(sc[:, :, delta:delta + SK],
                                     id_sb[:], msk_sb[:, :, db:db + SK],
                                     start=False, stop=True,
                                     skip_group_check=True)
                et = ep.tile([128, 2, SQ], bf16, tag="et", name=f"et_{c}_{j}")
                nc.scalar.activation(et[:, :, dd:SQ], sc[:, :, dd:SQ],
                                     FT.Exp, scale=0.125)
                pend.append((c, j, et, dd))

            def flush():
                c_, j_, et_, dd_ = pend.pop(0)
                for h in (0, 1):
                    nc.tensor.matmul(
                        avs[c_][h][:, dd_:SQ], vt[:, j_, 0:HD + 1],
                        et_[:, h, dd_:SQ],
                        start=(j_ == 0), stop=(j_ == 4 * (c_ + 1) - 1))

            def stage_av(n, av, ev):
                au0_t = aup.tile([HD + 1, SQ], f32r, tag="au0", name=f"au0_{n}")
                au1_t = aup.tile([HD + 1, SQ], f32r, tag="au1", name=f"au1_{n}")
                if ev == "s":
                    nc.scalar.activation(au0_t[:], av[0][:], FT.Copy)
                else:
                    nc.vector.tensor_copy(au0_t[:], av[0][:])
                nc.vector.tensor_copy(au1_t[:], av[1][:])
                aus[n] = (au0_t, au1_t)

            def wo_pair(c, p, evac, pool=None):
                sk0 = c * SQ
                pool = pool or pp
                tg = "pp2" if pool is pp else "sc"
                pw = pool.tile([128, 2, SQ], f32, tag=tg, name=f"pw_{c}_{p}")
                for i in (0, 1):
                    m = 2 * p + i
                    nc.tensor.matmul(pw[:, i, :], wo_sb[:, 128 * m:128 * (m + 1)],
                                     attSs[c][:, :], start=True, stop=True)
                ow = owp.tile([128, 2, SQ], bf16, tag="ow", name=f"ow_{c}_{p}")
                if evac == "s":
                    nc.scalar.activation(ow[:, :, :], pw[:, :, :], FT.Copy)
                else:
                    nc.vector.tensor_copy(ow[:, :, :], pw[:, :, :])
                eng = nc.sync if p % 2 == 0 else nc.scalar
                eng.dma_start(out_d[:, 2 * p:2 * p + 2, sk0:sk0 + SQ], ow[:, :, :])

            # ---------------- prologue ----------------
            # sync/gpsimd queues feed proj(0,1) x-tiles; scalar carries
            # consts + trig(0..2) + xt(2); the ones column is a DVE memset.
            nc.sync.dma_start(wq_sb[:, 0:256], wq_l[:, 0:256])
            nc.gpsimd.dma_start(wkv_sb[:, 0:256], wkv_l[:, 0:256])
            # chunks 0+1 as paired [128,2,512] tiles: 2KB descriptor rows
            # double the effective per-queue DMA throughput
            q3 = [nc.sync, nc.gpsimd, nc.scalar]
            xts[0] = []
            xts[1] = []
            for r in range(NR):
                if r == 2:
                    nc.sync.dma_start(wq_sb[:, 256:DIM], wq_l[:, 256:DIM])
                    nc.gpsimd.dma_start(wkv_sb[:, 256:DIM], wkv_l[:, 256:DIM])
                tp_ = xp.tile([128, 2, SQ], bf16, tag="xt", name=f"xtp_{r}")
                q3[r % 3].dma_start(tp_[:, :, :],
                                    xT[128 * r:128 * (r + 1), 0:2 * SQ])
                xts[0].append(tp_[:, 0, :])
                xts[1].append(tp_[:, 1, :])
            nc.scalar.dma_start(cos_sb[:, 0:3 * SQ], cos4_d[:, 0:3 * SQ])
            nc.scalar.dma_start(sin_sb[:, 0:3 * SQ], sin4_d[:, 0:3 * SQ])
            nc.scalar.dma_start(id_sb[:], id_d[:])
            nc.scalar.dma_start(psw_sb[:], psw_d[:])
            nc.sync.dma_start(msk_sb[:, 0, :], mask_d[:, 0, :])
            nc.gpsimd.dma_start(msk_sb[:, 1, :], mask_d[:, 1, :])
            load_xt(2, [nc.scalar])
            nc.scalar.dma_start(wo_sb[:], wo_l[:])
            nc.scalar.dma_start(sel_sb[:], sel_d[:])
            nc.vector.memset(vt[:, :, HD:HD + 1], 1.0)

            # PE warm-up: junk matmuls on the first-arriving weight slice keep
            # the tensor engine continuously busy through the DMA-bound load
            # window so the p-state ramp reaches full clock before proj(0).
            wrm = sp.tile([128, 2, SQ], f32, tag="sc", name="wrm")
            for _ in range(40):
                nc.tensor.matmul(wrm[:, 0, 0:128], wq_sb[:, 0:128],
                                 wq_sb[0:128, 0:128], start=True, stop=True)

            pp0 = pp.tile([128, 2, SQ], f32, tag="pp2", name="pp2_0")
            projp[0] = (pp0[:, 0, :], pp0[:, 1, :])
            proj_half(0, *projp[0], 0)
            proj_half(0, *projp[0], 1)
            rope(0, *projp[0])
            boot0 = ap.tile([128, SQ], f32, tag="av0", name="boot0")
            boot1 = ap.tile([128, SQ], f32, tag="av1", name="boot1")
            projp[1] = (boot0[:, :], boot1[:, :])
            proj_half(1, *projp[1], 0)
            proj_half(1, *projp[1], 1)
            rope_fin(0)
            vtr(0)

            # ---------------- phases ----------------
            for n in range(NQ):
                s0 = n * SQ
                nsk = 4 * (n + 1)
                if n > 0:
                    vtr(n)
                if n + 1 < NQ:
                    rope(n + 1, *projp.pop(n + 1))
                if n + 3 < NQ:
                    load_xt(n + 3, [nc.sync])
                    t3 = (n + 3) * SQ
                    nc.sync.dma_start(cos_sb[:, t3:t3 + SQ], cos4_d[:, t3:t3 + SQ])
                    nc.gpsimd.dma_start(sin_sb[:, t3:t3 + SQ], sin4_d[:, t3:t3 + SQ])

                # attachment thunks keyed by group index (leftovers pile at end)
                attach = {}

                def att_at(j, th):
                    attach.setdefault(j, []).append(th)

                # interleave pieces spread evenly across the group stream so
                # the 2-deep sc/exp pipeline never drains behind a pile-up
                pieces = []
                if n == 0:
                    def fin1():
                        bq1 = pp.tile([128, 2, SQ], f32, tag="pp2", name="bq1")
                        rope_fin(1, bq1[:, 0, :])

                    pieces.append(fin1)
                elif n + 1 < NQ:
                    pieces.append(lambda v=n + 1: rope_fin(v))
                if n >= 1:
                    c = n - 1
                    ev = "s" if n <= 3 else "v"
                    pieces.append(lambda c=c: endgame_bc(c))
                    pieces.append(lambda c=c: wo_pair(c, 0, "v"))
                    pieces.append(lambda c=c: wo_pair(c, 1, "v"))
                    pieces.append(lambda c=c, ev=ev: wo_pair(c, 2, ev))
                    pieces.append(lambda c=c, ev=ev: wo_pair(c, 3, ev))
                m = n + 2
                if m < NQ:
                    def proj_q(m=m, q=0):
                        pt = pp.tile([128, 2, SQ], f32, tag="pp2", name=f"pp2_{m}")
                        projp[m] = (pt[:, 0, :], pt[:, 1, :])
                        proj_quarter(m, q)

                    pieces.append(proj_q)
                    for q in range(1, 4):
                        pieces.append(lambda m=m, q=q: proj_quarter(m, q))
                G = nsk - (STEAL[n - 1] if n else 0)
                lo = 1 if n == 0 else 3
                for k, th in enumerate(pieces):
                    pos = lo + (k * max(0, G - 1 - lo)) // max(1, len(pieces) - 1) \
                        if len(pieces) > 1 else lo
                    att_at(min(pos, G - 1), th)

                avs[n] = [ap.tile([HD + 1, SQ], f32, tag=f"av{h}",
                                  name=f"av{h}_{n}") for h in (0, 1)]

                for idx, j in enumerate(range(STEAL[n - 1] if n else 0, nsk)):
                    sgrp(n, j)
                    if len(pend) > 2:
                        flush()
                    for th in attach.pop(idx, []):
                        th()
                while pend:
                    flush()

                stage_av(n, avs[n], "s" if (n <= 3 or n == NQ - 1) else "v")

                # leftovers (small phases): preserve order
                for idx in sorted(attach):
                    for th in attach[idx]:
                        th()

                # steal: next chunk's first score groups + exps run here,
                # filling this phase's scalar idle; their AV flushes happen
                # in the next phase once its accumulator bank opens.
                if n + 1 < NQ:
                    for j in range(STEAL[n]):
                        sgrp(n + 1, j)

            # ---------------- tail: endgame of last chunk ----------------
            # wo pairs alternate between the pp2 and (now idle) sc PSUM
            # rotations and between DVE/scalar evacuation so they pipeline.
            c = NQ - 1
            endgame_bc(c)
            wo_pair(c, 0, "v")
            wo_pair(c, 1, "s", pool=sp)
            wo_pair(c, 2, "v")
            wo_pair(c, 3, "s", pool=sp)


def _build():
    if "nc" in _CACHE:
        return _CACHE["nc"]
    nc = bacc.Bacc("TRN2", target_bir_lowering=False, debug=False, num_devices=NCORES)
    _emit(nc)
    nc.compile()
    _CACHE["nc"] = nc
    return nc


def _host_inputs(x, freqs_cos, freqs_sin, wq, wk, wv, wo):
    x = np.asarray(x, np.float32)
    freqs_cos = np.asarray(freqs_cos, np.float32)
    freqs_sin = np.asarray(freqs_sin, np.float32)
    wq = np.asarray(wq, np.float32)
    wk = np.asarray(wk, np.float32)
    wv = np.asarray(wv, np.float32)
    wo = np.asarray(wo, np.float32)

    xT = np.ascontiguousarray(x[0].T).astype(ml_dtypes.bfloat16)   # [1024, 4096]
    cosT = freqs_cos.T                                             # [32, 4096]
    sinT = freqs_sin.T
    cos4 = np.ascontiguousarray(np.tile(cosT, (4, 1))).astype(
        ml_dtypes.bfloat16)                                        # [128, 4096]
    sin4 = np.ascontiguousarray(
        np.concatenate([sinT, -sinT, sinT, -sinT], axis=0)).astype(
        ml_dtypes.bfloat16)

    # q rotate-half partition permutation (swap 32-row blocks in each half)
    pswap = np.zeros((128, 128), dtype=ml_dtypes.bfloat16)
    for mm in range(128):
        kk = mm + 32 if (mm % 64) < 32 else mm - 32
        pswap[kk, mm] = 1.0

    # diagonal-block causal masks for delta in {0,128,256,384}
    p = np.arange(SK)[:, None]
    f = np.arange(SQ)[None, :]
    mask = np.concatenate(
        [np.where(SK * d + p <= f, 0.0, MASKVAL) for d in range(4)],
        axis=1).astype(ml_dtypes.bfloat16)                         # [128, 2048]
    mask = np.ascontiguousarray(
        np.repeat(mask[:, None, :], 2, axis=1))                    # [128, 2, 2048]

    ident = np.eye(128, dtype=ml_dtypes.bfloat16)
    # two K=1 selector rows at partition 64: cols 0:128 broadcast head-0's
    # denominator to partitions 0:64, cols 128:256 head-1's to 64:128
    selr = np.zeros((128, 256), dtype=np.float32)
    selr[64, 0:64] = 1.0
    selr[64, 192:256] = 1.0

    perm = np.concatenate([np.arange(0, HD, 2), np.arange(1, HD, 2)])

    def fold(w):  # [128(m), 1024(d)] -> lhsT layout [128(p), 8r*128+m]
        return np.ascontiguousarray(
            w.reshape(128, NR, 128).transpose(2, 1, 0).reshape(128, DIM)
        ).astype(ml_dtypes.bfloat16)

    in_maps = []
    for c in range(NCORES):
        g = c // 2
        wq_c = wq[128 * c:128 * (c + 1)].reshape(2, HD, DIM)[:, perm, :].reshape(128, DIM)
        wk_g = wk[HD * g:HD * (g + 1)][perm]
        wv_g = wv[HD * g:HD * (g + 1)]
        wkv_c = np.concatenate([wv_g, wk_g], axis=0)        # v rows 0:64, k rows 64:128
        wo_c = np.ascontiguousarray(wo[:, 128 * c:128 * (c + 1)].T).astype(
            ml_dtypes.bfloat16)                              # [128(j), 1024(o)]
        in_maps.append({
            "xT": xT,
            "wq_l": fold(wq_c),
            "wkv_l": fold(wkv_c),
            "wo_l": wo_c,
            "cos4": cos4,
            "sin4": sin4,
            "mask": mask,
            "ident": ident,
            "pswap": pswap,
            "selr": selr,
        })
    return in_maps


def kernel(x, freqs_cos, freqs_sin, wq, wk, wv, wo, _trace=False, _trace_kwargs=None):
    nc = _build()
    in_maps = _host_inputs(x, freqs_cos, freqs_sin, wq, wk, wv, wo)
    kw = {}
    if _trace:
        kw.update(trace=True, **(_trace_kwargs or {}))
    res = run_bass_kernel_spmd(nc, in_maps, core_ids=list(range(NCORES)), **kw)
    acc = np.zeros((128, NR, SEQ), np.float32)
    for c in range(NCORES):
        acc += res.results[c]["out"].astype(np.float32)
    full = np.ascontiguousarray(acc.transpose(1, 0, 2)).reshape(DIM, SEQ)
    out = np.ascontiguousarray(full.T).reshape(1, SEQ, DIM)
    if _trace:
        kernel._last_results = res
    return out
